# revision 1
# baseline (speedup 1.0000x reference)
"""Trainium2 Bass kernel: causal multi-head self-attention (b=2, s=2048, d=1024, h=16).

Distribution (8 NeuronCores, SPMD single program):
  - Tensor-parallel over heads: core c owns heads {2c, 2c+1}. It computes those
    heads' Q/K/V projections over the full sequence (needs full x, its 128-col
    slices of Wq/Wk/Wv), then causal attention for its heads. Causality is
    exploited at compile time (identical tile structure on every core — heads
    are symmetric, so there is no load imbalance and no dynamic control flow).
  - AllToAll redistributes the attention output from head-sharded [all rows,
    128 cols] to row-sharded [512 rows, all 1024 cols].
  - Output projection is row-parallel: each core computes its 512 rows of
    attn @ Wo (full Wo on every core). Host just concatenates.

Compute dtype bf16 (PSUM accumulation fp32), weights/activations cast on host.
Softmax is computed without max-subtraction (scores are O(5) for this
distribution; exp is safe in fp32/bf16) which makes the flash accumulation a
plain sum. The denominator comes for free as a 65th "ones" column appended to
V; normalization is fused into the PSUM->SBUF copy as an ACT per-partition
scale.
"""

import sys

for _p in ("/opt/trn_rl_repo",):
    if _p not in sys.path:
        sys.path.insert(0, _p)

import numpy as np
import ml_dtypes

import concourse.bass as bass
import concourse.mybir as mybir
import concourse.tile as tile
from concourse import bacc
from concourse.bass_utils import run_bass_kernel_spmd

BF16 = mybir.dt.bfloat16
F32 = mybir.dt.float32
AF = mybir.ActivationFunctionType

B, S, D, H, DK = 2, 2048, 1024, 16, 64
NROWS = B * S          # 4096 flattened (batch, seq) rows
NC = 8                 # cores
HPC = H // NC          # 2 heads per core
DHC = HPC * DK         # 128 head-dim columns per core
RPC = NROWS // NC      # 512 output rows per core
QB = 16                # 128-row query blocks per batch
SCALE = 1.0 / float(np.sqrt(DK))


def _build_kernel(nc: bass.Bass, single_core: bool = False):
    xT = nc.dram_tensor("xT", [D, NROWS], BF16, kind="ExternalInput")
    wq = nc.dram_tensor("wq", [D, DHC], BF16, kind="ExternalInput")
    wk = nc.dram_tensor("wk", [D, DHC], BF16, kind="ExternalInput")
    wv = nc.dram_tensor("wv", [D, DHC], BF16, kind="ExternalInput")
    wo = nc.dram_tensor("wo", [D, D], BF16, kind="ExternalInput")
    maskin = nc.dram_tensor("maskin", [128, 128], BF16, kind="ExternalInput")
    identin = nc.dram_tensor("identin", [128, 128], BF16, kind="ExternalInput")
    out = nc.dram_tensor("out", [RPC, D], F32, kind="ExternalOutput")

    with tile.TileContext(nc) as tc:
        _body(tc, xT, wq, wk, wv, wo, maskin, identin, out, single_core)


def _body(tc, xT, wq, wk, wv, wo, maskin, identin, out, single_core=False):
    nc = tc.nc
    from contextlib import ExitStack

    with ExitStack() as ctx:
        const_pool = ctx.enter_context(tc.tile_pool(name="const", bufs=1))
        proj_pool = ctx.enter_context(tc.tile_pool(name="proj", bufs=1))
        x_pool = ctx.enter_context(tc.tile_pool(name="x", bufs=3))
        w_pool = ctx.enter_context(tc.tile_pool(name="w", bufs=1))
        psum_pool = ctx.enter_context(
            tc.tile_pool(name="psum", bufs=2, space="PSUM")
        )
        st_pool = psum_pool
        acc_pool = psum_pool
        sb_pool = ctx.enter_context(tc.tile_pool(name="sb", bufs=4))
        dram_pool = ctx.enter_context(
            tc.tile_pool(name="dram", bufs=1, space="DRAM")
        )

        # ---- weights + constants ----------------------------------------
        # [D, M] -> sbuf [128, D//128, M] (partition = din % 128).
        # DMAs spread across engine queues so the first projection's inputs
        # (wq + first x group) aren't stuck behind the rest.
        wq_sb = w_pool.tile([128, 8, DHC], BF16, tag="wq")
        wk_sb = w_pool.tile([128, 8, DHC], BF16, tag="wk")
        wv_sb = w_pool.tile([128, 8, DHC], BF16, tag="wv")
        wo_sb = w_pool.tile([128, 8, D], BF16, tag="wo")
        nc.sync.dma_start(wq_sb[:], wq.ap().rearrange("(c p) m -> p c m", p=128))
        nc.scalar.dma_start(wk_sb[:], wk.ap().rearrange("(c p) m -> p c m", p=128))
        nc.scalar.dma_start(wv_sb[:], wv.ap().rearrange("(c p) m -> p c m", p=128))
        mask_sb = const_pool.tile([128, 128], BF16)
        nc.gpsimd.dma_start(mask_sb[:], maskin[:, :])
        ident_sb = const_pool.tile([128, 128], BF16)
        nc.gpsimd.dma_start(ident_sb[:], identin[:, :])

        # ---- projections: qT/kT/vT [128 (2 heads x 64), 4096] bf16 -----
        # pair index = hl*2 + b (hl-major to enable the split all-to-all)
        qT = proj_pool.tile([128, NROWS], BF16, tag="qT")
        kT = proj_pool.tile([128, NROWS], BF16, tag="kT")
        vT = proj_pool.tile([128, NROWS], BF16, tag="vT")
        v_aug = proj_pool.tile([128, 4, QB, DK + 1], BF16, tag="vaug")
        xT_r = xT.ap().rearrange("(c p) n -> p c n", p=128)

        def build_vaug(hl, b, c0s=None):
            pair = hl * 2 + b
            hs = hl * DK
            if c0s is None or 0 in c0s:
                nc.vector.memset(v_aug[:, pair, :, DK : DK + 1], 1.0)
            for c0 in c0s if c0s is not None else range(0, QB, 8):
                pt = st_pool.tile([128, 8, DK], BF16, tag="mm512")
                for ci in range(8):
                    col0 = b * S + (c0 + ci) * 128
                    nc.tensor.transpose(
                        pt[:, ci, :],
                        vT[hs : hs + DK, col0 : col0 + 128],
                        ident_sb[hs : hs + DK, hs : hs + DK],
                    )
                nc.any.tensor_copy(
                    v_aug[:, pair, c0 : c0 + 8, 0:DK], pt[:]
                )

        def proj_group(g):
            xg = x_pool.tile([128, 8, 512], BF16, tag="xg")
            nc.sync.dma_start(xg[:], xT_r[:, :, g * 512 : (g + 1) * 512])
            for w_sb, projT in ((wq_sb, qT), (wk_sb, kT), (wv_sb, vT)):
                ps = psum_pool.tile([128, 512], F32, tag="mm512")
                for dc in range(8):
                    nc.tensor.matmul(
                        ps[:],
                        w_sb[:, dc, :],
                        xg[:, dc, :],
                        start=(dc == 0),
                        stop=(dc == 7),
                    )
                nc.any.tensor_copy(
                    projT[:, g * 512 : (g + 1) * 512], ps[:]
                )

        # ---- attention -------------------------------------------------
        # Per (pair, qgroup of 512): S^T chunks [128 k, 512 q] in PSUM, exp
        # on ACT, diagonal-band masking on DVE, PV with V stationary into a
        # transposed accumulator acc_T [65, 512] (row 64 = softmax denom).
        # send_buf[dest, :, :] = [128 d-rows (2 heads), 512 q] slab.
        # one contiguous buffer pair per head-half (collectives require
        # contiguous access patterns)
        send_h = [
            dram_pool.tile(
                [NC, DK, RPC], BF16, tag=f"send{hl}", name=f"send_h{hl}"
            )
            for hl in range(2)
        ]
        recv_h = [
            dram_pool.tile(
                [NC, DK, RPC], BF16, tag=f"recv{hl}", name=f"recv_h{hl}"
            )
            for hl in range(2)
        ]

        def attend_group(hl, b, g):
            pair = hl * 2 + b
            hs = hl * DK
            if True:
                qcol0 = b * S + g * 512
                nck = 4 * g + 4
                acc = acc_pool.tile([DK + 1, 512], F32, tag="acc")
                for ci in range(0, nck, 2):
                    st = st_pool.tile([128, 2, 512], F32, tag="st")
                    if ci + 2 <= 4 * g:
                        # below the diagonal band: full-width, batched exp
                        for j in range(2):
                            kcol0 = b * S + (ci + j) * 128
                            nc.tensor.matmul(
                                st[:, j, :],
                                kT[hs : hs + DK, kcol0 : kcol0 + 128],
                                qT[hs : hs + DK, qcol0 : qcol0 + 512],
                                start=True,
                                stop=True,
                            )
                        p_t = sb_pool.tile([128, 2, 512], BF16, tag="pt")
                        nc.scalar.activation(
                            p_t[:, :, :], st[:, :, :], AF.Exp, scale=SCALE
                        )
                        for j in range(2):
                            ck = ci + j
                            nc.tensor.matmul(
                                acc[:],
                                v_aug[:, pair, ck, :],
                                p_t[:, j, :],
                                start=(ck == 0),
                                stop=False,
                            )
                    else:
                        # diagonal band: only q columns >= r*128 are live.
                        # One exp covers both chunks' suffixes (the union
                        # starts at the first chunk's offset; the extra
                        # columns of the second chunk are never read).
                        p_t = sb_pool.tile([128, 2, 512], BF16, tag="pt")
                        r0 = ci - 4 * g
                        c0u = r0 * 128
                        for j in range(2):
                            ck = ci + j
                            kcol0 = b * S + ck * 128
                            nc.tensor.matmul(
                                st[:, j, c0u:512],
                                kT[hs : hs + DK, kcol0 : kcol0 + 128],
                                qT[
                                    hs : hs + DK,
                                    qcol0 + c0u : qcol0 + 512,
                                ],
                                start=True,
                                stop=True,
                            )
                        nc.scalar.activation(
                            p_t[:, :, r0 * 128 : 512],
                            st[:, :, r0 * 128 : 512],
                            AF.Exp,
                            scale=SCALE,
                        )
                        for j in range(2):
                            ck = ci + j
                            r = ck - 4 * g
                            c0 = r * 128
                            nc.vector.tensor_mul(
                                p_t[:, j, c0 : c0 + 128],
                                p_t[:, j, c0 : c0 + 128],
                                mask_sb[:],
                            )
                            nc.tensor.matmul(
                                acc[:, c0:512],
                                v_aug[:, pair, ck, :],
                                p_t[:, j, c0:512],
                                start=(ck == 0),
                                stop=(ck == nck - 1),
                            )
                recip = sb_pool.tile([1, 512], F32, tag="recip")
                nc.vector.reciprocal(recip[:], acc[DK : DK + 1, :])
                bcast = sb_pool.tile([DK, 512], F32, tag="bcast")
                nc.gpsimd.partition_broadcast(bcast[:], recip[:])
                slab = sb_pool.tile([DK, 512], BF16, tag="slab")
                nc.vector.tensor_mul(slab[:], acc[0:DK, :], bcast[:])
                dest = b * 4 + g
                nc.sync.dma_start(send_h[hl][dest, :, :], slab[:])

        def a2a(hl):
            if single_core:
                nc.sync.dma_start(recv_h[hl][:], send_h[hl][:])
            else:
                nc.gpsimd.collective_compute(
                    "AllToAll",
                    mybir.AluOpType.bypass,
                    replica_groups=[list(range(NC))],
                    ins=[send_h[hl].opt()],
                    outs=[recv_h[hl].opt()],
                )

        # Emission order engineered for overlap: batch-0 projections first,
        # then batch-1 projections woven with batch-0/head-0 attention (keeps
        # ACT busy while PE does projections), then the remaining pairs with
        # the first all-to-all (hl=0 rows) overlapping hl=1 attention.
        for g in range(4):
            proj_group(g)
        build_vaug(0, 0)
        build_vaug(1, 0)
        proj_group(4)
        attend_group(0, 0, 0)
        attend_group(1, 0, 0)
        proj_group(5)
        attend_group(0, 0, 1)
        attend_group(1, 0, 1)
        build_vaug(0, 1, c0s=[0])
        build_vaug(1, 1, c0s=[0])
        proj_group(6)
        attend_group(0, 0, 2)
        attend_group(1, 0, 2)
        attend_group(0, 1, 0)
        proj_group(7)
        attend_group(0, 0, 3)
        attend_group(1, 0, 3)
        build_vaug(0, 1, c0s=[8])
        build_vaug(1, 1, c0s=[8])
        attend_group(0, 1, 1)
        nc.scalar.dma_start(
            wo_sb[:], wo.ap().rearrange("(c p) m -> p c m", p=128)
        )
        attnT = proj_pool.tile([128, 8, RPC], BF16, tag="attnT")

        def gather(hl):
            hs = hl * DK
            for src in range(NC):
                nc.scalar.dma_start(
                    attnT[hs : hs + DK, src, :], recv_h[hl][src, :, :]
                )

        for g in range(2, 4):
            attend_group(0, 1, g)
        a2a(0)
        gather(0)
        for g in range(4):
            attend_group(1, 1, g)
        a2a(1)
        gather(1)

        # ---- output projection: out rows [512, 1024] f32 ---------------
        for qb in range(4):
            orow = sb_pool.tile([128, D], F32, tag="orow")
            for half in range(2):
                po = psum_pool.tile([128, 512], F32, tag="mm512")
                for dc in range(8):
                    nc.tensor.matmul(
                        po[:],
                        attnT[:, dc, qb * 128 : (qb + 1) * 128],
                        wo_sb[:, dc, half * 512 : (half + 1) * 512],
                        start=(dc == 0),
                        stop=(dc == 7),
                    )
                nc.any.tensor_copy(
                    orow[:, half * 512 : (half + 1) * 512], po[:]
                )
            nc.sync.dma_start(out[qb * 128 : (qb + 1) * 128, :], orow[:])


_CACHE = {}


def _get_compiled():
    if "nc" not in _CACHE:
        nc = bacc.Bacc(
            "TRN2", target_bir_lowering=False, debug=False, num_devices=NC
        )
        _build_kernel(nc)
        nc.compile()
        _CACHE["nc"] = nc
    return _CACHE["nc"]


def _make_in_maps(x, Wq, Wk, Wv, Wo):
    bf = ml_dtypes.bfloat16
    xT = np.ascontiguousarray(
        x.reshape(NROWS, D).T.astype(bf)
    )  # [D, NROWS]
    wo = np.ascontiguousarray(Wo.astype(bf))
    # mask[k, q] = 1 where k <= q (allowed) for a diagonal 128x128 tile
    mask = np.triu(np.ones((128, 128), dtype=np.float32)).astype(bf)
    ident = np.eye(128, dtype=np.float32).astype(bf)
    in_maps = []
    for c in range(NC):
        sl = slice(c * DHC, (c + 1) * DHC)
        in_maps.append(
            {
                "xT": xT,
                "wq": np.ascontiguousarray(Wq[:, sl].astype(bf)),
                "wk": np.ascontiguousarray(Wk[:, sl].astype(bf)),
                "wv": np.ascontiguousarray(Wv[:, sl].astype(bf)),
                "wo": wo,
                "maskin": mask,
                "identin": ident,
            }
        )
    return in_maps


def _get_runner():
    """Build (once) a cached jitted SPMD executor mirroring
    concourse.bass2jax.run_bass_via_pjrt's multi-core path, so repeat calls
    skip retracing/recompiling the wrapper."""
    if "runner" in _CACHE:
        return _CACHE["runner"]
    import jax
    from jax.sharding import Mesh, PartitionSpec
    from jax.experimental.shard_map import shard_map
    from concourse import bass2jax

    nc = _get_compiled()
    bass2jax.install_neuronx_cc_hook()
    in_names, out_names, out_avals, zero_shapes = [], [], [], []
    partition_name = (
        nc.partition_id_tensor.name if nc.partition_id_tensor else None
    )
    for alloc in nc.m.functions[0].allocations:
        if not isinstance(alloc, mybir.MemoryLocationSet):
            continue
        name = alloc.memorylocations[0].name
        if alloc.kind == "ExternalInput":
            if name != partition_name:
                in_names.append(name)
        elif alloc.kind == "ExternalOutput":
            shape = tuple(alloc.tensor_shape)
            dtype = mybir.dt.np(alloc.dtype)
            out_names.append(name)
            out_avals.append(jax.core.ShapedArray(shape, dtype))
            zero_shapes.append((shape, dtype))
    n_params = len(in_names)
    all_names = in_names + out_names
    if partition_name is not None:
        all_names = all_names + [partition_name]
    all_in_names = tuple(all_names)

    def _body(*args):
        operands = list(args)
        if partition_name is not None:
            operands.append(bass2jax.partition_id_tensor())
        return tuple(
            bass2jax._bass_exec_p.bind(
                *operands,
                out_avals=tuple(out_avals),
                in_names=all_in_names,
                out_names=tuple(out_names),
                lowering_input_output_aliases=(),
                sim_require_finite=True,
                sim_require_nnan=True,
                nc=nc,
            )
        )

    devices = jax.devices()[:NC]
    mesh = Mesh(np.asarray(devices), ("core",))
    nin = n_params + len(out_names)
    sharded = jax.jit(
        shard_map(
            _body,
            mesh=mesh,
            in_specs=(PartitionSpec("core"),) * nin,
            out_specs=(PartitionSpec("core"),) * len(out_names),
            check_rep=False,
        ),
        donate_argnums=tuple(range(n_params, nin)),
        keep_unused=True,
    )

    def run(in_maps):
        concat_in = [
            np.concatenate(
                [np.asarray(in_maps[c][nm]) for c in range(NC)], axis=0
            )
            for nm in in_names
        ]
        concat_zeros = [
            np.zeros((NC * s[0], *s[1:]), dt) for s, dt in zero_shapes
        ]
        out_arrs = sharded(*concat_in, *concat_zeros)
        return [
            {
                name: np.asarray(out_arrs[i]).reshape(
                    NC, *out_avals[i].shape
                )[c]
                for i, name in enumerate(out_names)
            }
            for c in range(NC)
        ]

    _CACHE["runner"] = run
    return run


def kernel(x, Wq, Wk, Wv, Wo, _run_kwargs=None):
    x = np.asarray(x, dtype=np.float32)
    in_maps = _make_in_maps(np.asarray(x), np.asarray(Wq), np.asarray(Wk),
                            np.asarray(Wv), np.asarray(Wo))
    if _run_kwargs:
        nc = _get_compiled()
        res = run_bass_kernel_spmd(
            nc, in_maps, core_ids=list(range(NC)), **_run_kwargs
        )
        _CACHE["last_results"] = res
        results = res.results
    else:
        results = _get_runner()(in_maps)
    outs = [results[c]["out"] for c in range(NC)]
    full = np.concatenate(outs, axis=0)  # [4096, 1024]
    return full.reshape(B, S, D).astype(np.float32)

